# revision 15
# baseline (speedup 1.0000x reference)
"""GAT-style graph encoder on 8 trn2 NeuronCores — v3.

Reference (per exercise i over kc nodes j):
    kc_Wh = kc_h @ W1; ex_Wh = ex_h @ W1
    e[i,j] = leaky_relu(u_i + v_j, 0.2),  u = ex_Wh@a1, v = kc_Wh@a2
    att = softmax(where(adj>0, e, -9e15), axis=1)
    new_kc = att @ kc_Wh; ex_Eh = ex_h @ E
    out = elu(concat([new_kc, new_kc*ex_Eh]) @ rd_w.T + rd_b)

Strategy (row-shard exercises over 8 cores, 1250 rows -> padded 1280):
The pre-activation logit is separable (u_i + v_j), so with the softmax shift
r_i = u_i + c (softmax is invariant to any per-row scale) the masked exp
factors into rank-1 products:
    p[j,i] = adj * max(C'_j, D_j * B'_i),  C' = e^{v-c}, D = e^{0.2 v},
    B' = e^{-0.8 u - c}    (all host-computed rows; exact algebra).
kc nodes are host-sorted by v (descending), exercises are host-sorted by u
(descending, per core).  Then for each kc chunk there is a column prefix
t_kk = #{i : u_i >= -min_j v_j} where the positive branch wins for EVERY
(j,i) pair, i.e. p = adj * C'_j exactly.  For that prefix the aggregation is
a plain matmul with adj itself as the moving tensor and kcWh*C' folded into
the stationary - no elementwise work at all.  Only the column suffix needs
the two elementwise passes (a 4x DVE tensor_scalar for q and a mask multiply
split across DVE/Pool).  The 4 top (high-v) chunks aggregate in bf16; the 12
tail chunks use fp8e4 DoubleRow matmuls (2 k-tiles/instr at 0.5 cyc/row).
Readout runs in bf16.  The per-row softmax division, +rd_b and elu are
applied on the host during unshard (per-row scalar epilogue).
"""

import ml_dtypes
import numpy as np

import concourse.bacc as bacc
import concourse.bass as bass
import concourse.mybir as mybir
from concourse.alu_op_type import AluOpType
from concourse.bass_utils import run_bass_kernel_spmd
from concourse.tile import TileContext

F32 = mybir.dt.float32
BF16 = mybir.dt.bfloat16
FP8 = mybir.dt.float8e4
DR = mybir.MatmulPerfMode.DoubleRow

P = 128
D = 256
NKC = 2048
KCH = 16                    # kc chunks
NBF = 4                     # leading (high-v) chunks aggregated in bf16
NPAIR = (KCH - NBF) // 2    # fp8 DoubleRow chunk pairs
M = 1280                    # padded exercise rows per core
MBS = (512, 512, 256)
MOFF = (0, 512, 1024)
NCORES = 8
ROWS = 1250
N_E = 10000
SCALE = 128.0               # fp8 range scale folded into B'/C' (cancels in n/s)
# tail chunks whose suffix mask multiply runs on DVE (fp8 out, 1x) vs Pool
DVE_MASK = frozenset((4, 5, 6, 7, 8, 9, 10, 11))

NP_BF16 = ml_dtypes.bfloat16
NP_FP8 = ml_dtypes.float8_e4m3


def _build(Ts):
    """Ts: per-chunk column counts (multiple of 64) where p = adj*C' exactly."""
    nc = bacc.Bacc("TRN2", target_bir_lowering=False, debug=False,
                   num_devices=NCORES)
    adjg = [nc.declare_dram_parameter(f"adjg{g}", [P, 4 * M],
                                      FP8, isOutput=False) for g in range(4)]
    exTb = nc.declare_dram_parameter("exTb", [P, 2 * M], BF16, isOutput=False)
    kcWhT8 = nc.declare_dram_parameter("kcWhT8", [P, NPAIR * 512], FP8,
                                       isOutput=False)
    kcCT8 = nc.declare_dram_parameter("kcCT8", [P, NPAIR * 512], FP8,
                                      isOutput=False)
    sC8 = nc.declare_dram_parameter("sC8", [P, NPAIR * 32], FP8,
                                    isOutput=False)
    kcWhTb = nc.declare_dram_parameter("kcWhTb", [P, NBF * 256], BF16,
                                       isOutput=False)
    kcCTb = nc.declare_dram_parameter("kcCTb", [P, NBF * 256], BF16,
                                      isOutput=False)
    sCb = nc.declare_dram_parameter("sCb", [P, NBF], BF16, isOutput=False)
    EmB = nc.declare_dram_parameter("EmB", [P, 2 * 256], BF16, isOutput=False)
    rdwB = nc.declare_dram_parameter("rdwB", [P, 4 * 256], BF16,
                                     isOutput=False)
    rowB = nc.declare_dram_parameter("rowB", [1, M], BF16, isOutput=False)
    scal = nc.declare_dram_parameter("scal", [P, 32], F32, isOutput=False)
    outB = nc.declare_dram_parameter("outB", [P, 2 * M], BF16, isOutput=True)
    srow = nc.declare_dram_parameter("srow", [1, M], F32, isOutput=True)

    Tpair = [min(Ts[NBF + 2 * pr], Ts[NBF + 2 * pr + 1])
             for pr in range(NPAIR)]

    with TileContext(nc) as tc:
        with tc.tile_pool(name="const", bufs=1) as cpool, \
             tc.tile_pool(name="acc_ps", bufs=2, space="PSUM") as apool, \
             tc.tile_pool(name="out_ps", bufs=2, space="PSUM") as opool, \
             tc.tile_pool(name="mwork", bufs=4) as mpool, \
             tc.tile_pool(name="post", bufs=2) as qpool:
            # ---- const loads: q-gating rows, then aggregation stationaries
            # and adj groups (unblock PE A-matmuls early), then post-stage data
            scal_sb = cpool.tile([P, 32], F32, tag="scal")
            nc.sync.dma_start(out=scal_sb[:], in_=scal[:, :])
            rowB_sb = cpool.tile([1, M], BF16, tag="rowB")
            nc.sync.dma_start(out=rowB_sb[:], in_=rowB[:, :])
            kcCb_sb = cpool.tile([P, NBF * 256], BF16, tag="kcCTb")
            nc.sync.dma_start(out=kcCb_sb[:], in_=kcCTb[:, :])
            kcb_sb = cpool.tile([P, NBF * 256], BF16, tag="kcWhTb")
            nc.sync.dma_start(out=kcb_sb[:], in_=kcWhTb[:, :])
            sCb_sb = cpool.tile([P, NBF], BF16, tag="sCb")
            nc.sync.dma_start(out=sCb_sb[:], in_=sCb[:, :])
            adj_sb = []
            for g in range(4):
                t = cpool.tile([P, 4, M], FP8, tag=f"adjg{g}", name=f"adjg{g}")
                nc.sync.dma_start(out=t[:], in_=adjg[g][:, :])
                adj_sb.append(t)
                if g == 0:
                    # 4-D: [p, (pair,target), ktile=2, m] for DoubleRow APs
                    kc8_sb = cpool.tile([P, NPAIR * 2, 2, P], FP8, tag="kcWhT8")
                    nc.sync.dma_start(out=kc8_sb[:], in_=kcWhT8[:, :])
                    kcC8_sb = cpool.tile([P, NPAIR * 2, 2, P], FP8, tag="kcCT8")
                    nc.sync.dma_start(out=kcC8_sb[:], in_=kcCT8[:, :])
                    sC8_sb = cpool.tile([P, NPAIR, 2, 16], FP8, tag="sC8")
                    nc.sync.dma_start(out=sC8_sb[:], in_=sC8[:, :])
                if g == 1:
                    exT_sb = cpool.tile([P, 2 * M], BF16, tag="exTb")
                    nc.sync.dma_start(out=exT_sb[:], in_=exTb[:, :])
                    EmB_sb = cpool.tile([P, 2 * 256], BF16, tag="EmB")
                    nc.sync.dma_start(out=EmB_sb[:], in_=EmB[:, :])
                if g == 2:
                    rdw_sb = cpool.tile([P, 4 * 256], BF16, tag="rdwB")
                    nc.sync.dma_start(out=rdw_sb[:], in_=rdwB[:, :])

            ones1b = cpool.tile([1, P], BF16, tag="ones1b")
            nc.vector.memset(ones1b[:], 1.0)
            onesb = cpool.tile([P, 1], BF16, tag="onesb")
            nc.vector.memset(onesb[:], 1.0)
            ones8 = cpool.tile([P, 2, 16], FP8, tag="ones8")
            nc.vector.memset(ones8[:], 1.0)

            Bb = cpool.tile([P, M], BF16, tag="Bb")          # B' broadcast
            exEhT = [cpool.tile([P, M], BF16, tag=f"exEhT{d}", name=f"exEhT{d}")
                     for d in (0, 1)]
            outB_sb = cpool.tile([P, 2 * M], BF16, tag="outB_sb")
            srow_sb = cpool.tile([1, M], F32, tag="srow_sb")

            # ---- setup: B' broadcast + ex_Eh (psum shared with readout pool)
            for b in range(3):
                ms = slice(MOFF[b], MOFF[b] + MBS[b])
                ps = opool.tile([P, MBS[b]], F32, tag="raw", name=f"bb_ps{b}")
                nc.tensor.matmul(ps[:], ones1b[:], rowB_sb[:, ms],
                                 start=True, stop=True)
                nc.scalar.copy(Bb[:, ms], ps[:])
            for d in range(2):
                for b in range(3):
                    ms = slice(MOFF[b], MOFF[b] + MBS[b])
                    ps = opool.tile([P, MBS[b]], F32, tag="raw",
                                    name=f"eh_ps{d}_{b}")
                    for c in range(2):
                        nc.tensor.matmul(
                            ps[:],
                            EmB_sb[:, c * 256 + d * P:c * 256 + (d + 1) * P],
                            exT_sb[:, c * M + MOFF[b]:c * M + MOFF[b] + MBS[b]],
                            start=(c == 0), stop=(c == 1))
                    nc.scalar.copy(exEhT[d][:, ms], ps[:])

            # ---- suffix-only masked-exp (cols >= T of each chunk)
            def adjsl(kk, lo, hi):
                g, o = divmod(kk, 4)
                return adj_sb[g][:, o, lo:hi]

            def q_of(kk, t0):
                q = mpool.tile([P, M], BF16, tag="q", bufs=4,
                               name=f"q{kk}")
                # (B'_i * D_j) max C'_j : whole unmasked exp in one 4x op
                nc.vector.tensor_scalar(
                    q[:, t0:], Bb[:, t0:], scal_sb[:, 16 + kk:17 + kk],
                    scal_sb[:, kk:kk + 1], AluOpType.mult, AluOpType.max)
                return q

            ptmb = []
            for kk in range(NBF):
                t0 = 0 if kk == 0 else Ts[kk]
                if t0 >= M:
                    ptmb.append(None)
                    continue
                q = q_of(kk, t0)
                t = mpool.tile([P, M], BF16, tag="ptmb", bufs=NBF,
                               name=f"ptmb{kk}")
                nc.vector.tensor_mul(t[:, t0:], q[:, t0:], adjsl(kk, t0, M))
                ptmb.append(t)
            ptm8 = []
            for pr in range(NPAIR):
                t0 = Tpair[pr]
                if t0 >= M:
                    ptm8.append(None)
                    continue
                t8 = mpool.tile([P, 2, M], FP8, tag="ptm8", bufs=NPAIR,
                                name=f"ptm8_{pr}")
                for h in range(2):
                    kk = NBF + 2 * pr + h
                    q = q_of(kk, t0)
                    eng = nc.vector if kk in DVE_MASK else nc.gpsimd
                    eng.tensor_mul(t8[:, h, t0:], q[:, t0:], adjsl(kk, t0, M))
                ptm8.append(t8)

            # ---- per m-block aggregation + readout
            for b in range(3):
                mb = MBS[b]
                ms = slice(MOFF[b], MOFF[b] + mb)
                n0 = apool.tile([P, mb], F32, tag="n0")
                n1 = apool.tile([P, mb], F32, tag="n1")
                sS = apool.tile([1, mb], F32, tag="sS")

                # bf16 chunks: A-prefix via adj-matmul, B-suffix via ptm.
                # start=True zeroes the whole 2KB psum bank, so the group
                # opener must be a full-width matmul emitted first: prefer a
                # chunk whose exact-C prefix covers the block (adj-only dep,
                # lets PE start without waiting for masks), else chunk 0's
                # full-width B path.
                starter = None
                for kk in range(1, NBF):
                    if Ts[kk] - MOFF[b] >= mb:
                        starter = kk
                        break
                order = ([starter] + [k for k in range(NBF) if k != starter]
                         if starter is not None else list(range(NBF)))
                for kk in order:
                    aw = 0 if kk == 0 else min(max(Ts[kk] - MOFF[b], 0), mb)
                    st = (kk == (order[0]))
                    if aw > 0:
                        asl = adjsl(kk, MOFF[b], MOFF[b] + aw)
                        nc.tensor.matmul(
                            n0[:, 0:aw], kcCb_sb[:, kk * 256:kk * 256 + P],
                            asl, start=st, stop=False, skip_group_check=True)
                        nc.tensor.matmul(
                            n1[:, 0:aw], kcCb_sb[:, kk * 256 + P:(kk + 1) * 256],
                            asl, start=st, stop=False, skip_group_check=True)
                        nc.tensor.matmul(
                            sS[:, 0:aw], sCb_sb[:, kk:kk + 1],
                            asl, start=st, stop=False, skip_group_check=True)
                    if aw < mb:
                        pm = ptmb[kk][:, MOFF[b] + aw:MOFF[b] + mb]
                        nc.tensor.matmul(
                            n0[:, aw:mb], kcb_sb[:, kk * 256:kk * 256 + P],
                            pm, start=st, stop=False, skip_group_check=True)
                        nc.tensor.matmul(
                            n1[:, aw:mb], kcb_sb[:, kk * 256 + P:(kk + 1) * 256],
                            pm, start=st, stop=False, skip_group_check=True)
                        nc.tensor.matmul(
                            sS[:, aw:mb], onesb[:],
                            pm, start=st, stop=False, skip_group_check=True)
                # fp8 DoubleRow pairs
                for pr in range(NPAIR):
                    aw = min(max(Tpair[pr] - MOFF[b], 0), mb)
                    g, o = divmod(NBF + 2 * pr, 4)
                    sp = (pr == NPAIR - 1)
                    if aw > 0:
                        adjpair = adj_sb[g][:, o:o + 2, MOFF[b]:MOFF[b] + aw]
                        nc.tensor.matmul(
                            n0[:, 0:aw], kcC8_sb[:, 2 * pr, :, :], adjpair,
                            start=False, stop=sp and aw >= mb,
                            perf_mode=DR, skip_group_check=True)
                        nc.tensor.matmul(
                            n1[:, 0:aw], kcC8_sb[:, 2 * pr + 1, :, :], adjpair,
                            start=False, stop=sp and aw >= mb,
                            perf_mode=DR, skip_group_check=True)
                        nc.tensor.matmul(
                            sS[:, 0:aw], sC8_sb[:, pr, :, 0:1], adjpair,
                            start=False, stop=sp and aw >= mb,
                            perf_mode=DR, skip_group_check=True)
                    if aw < mb:
                        pm = ptm8[pr][:, :, MOFF[b] + aw:MOFF[b] + mb]
                        nc.tensor.matmul(
                            n0[:, aw:mb], kc8_sb[:, 2 * pr, :, :], pm,
                            start=False, stop=sp, perf_mode=DR,
                            skip_group_check=True)
                        nc.tensor.matmul(
                            n1[:, aw:mb], kc8_sb[:, 2 * pr + 1, :, :], pm,
                            start=False, stop=sp, perf_mode=DR,
                            skip_group_check=True)
                        nc.tensor.matmul(
                            sS[:, aw:mb], ones8[:, :, 0:1], pm,
                            start=False, stop=sp, perf_mode=DR,
                            skip_group_check=True)

                # ---- post: features, readout, stage out
                nc.vector.tensor_copy(srow_sb[:, ms], sS[:])
                ncf = []
                for t in range(2):
                    nt = qpool.tile([P, mb], BF16, tag=f"nc{t}", name=f"nc{t}")
                    nc.scalar.copy(nt[:], (n0 if t == 0 else n1)[:])
                    ncf.append(nt)
                tf = []
                for t in range(2):
                    tt = qpool.tile([P, mb], BF16, tag=f"t{t}", name=f"tt{t}")
                    nc.vector.tensor_mul(tt[:], ncf[t][:], exEhT[t][:, ms])
                    tf.append(tt)
                feats = [ncf[0], ncf[1], tf[0], tf[1]]
                for oo in range(2):
                    raw = opool.tile([P, mb], F32, tag="raw")
                    for dd in range(4):
                        nc.tensor.matmul(
                            raw[:], rdw_sb[:, dd * 256 + oo * P:dd * 256 + (oo + 1) * P],
                            feats[dd][:], start=(dd == 0), stop=(dd == 3))
                    # stage to outB interleaved (col 2i+oo) for one DMA/block
                    nc.scalar.copy(
                        outB_sb[:, 2 * MOFF[b] + oo:2 * (MOFF[b] + mb):2],
                        raw[:])
                nc.sync.dma_start(
                    out=outB[:, 2 * MOFF[b]:2 * (MOFF[b] + mb)],
                    in_=outB_sb[:, 2 * MOFF[b]:2 * (MOFF[b] + mb)])
            nc.sync.dma_start(out=srow[:, :], in_=srow_sb[:])
    nc.finalize()
    return nc


_PROGRAMS = {}


def _get_program(Ts):
    key = tuple(Ts)
    if key not in _PROGRAMS:
        _PROGRAMS[key] = _build(key)
    return _PROGRAMS[key]


def _prep(exercise_h, kc_h, adj, W1, E, a, rd_w, rd_b):
    f = np.float32
    ex = np.asarray(exercise_h, dtype=np.float64)
    kc = np.asarray(kc_h, dtype=np.float64)
    W1 = np.asarray(W1, dtype=np.float64)
    E_ = np.asarray(E, dtype=np.float64)
    a = np.asarray(a, dtype=np.float64)
    a1, a2 = a[:D, 0], a[D:, 0]

    u = ex @ (W1 @ a1)                        # [N_E]
    vp = np.full(NKC, -60.0)
    vp[:kc.shape[0]] = kc @ (W1 @ a2)
    order = np.argsort(-vp, kind="stable")
    vs = vp[order]
    vmax = vs[0]
    c = float((np.maximum(u + vmax, 0.2 * (u + vmax)) - u).max())

    Brow = (SCALE * np.exp(-0.8 * u - c)).astype(f)            # [N_E]
    Cs = (SCALE * np.exp(vs - c)).astype(f)                    # [NKC]
    Ds = np.exp(0.2 * vs).astype(f)                            # [NKC]
    scal = np.zeros((P, 32), dtype=f)
    scal[:, :16] = Cs.reshape(KCH, P).T
    scal[:, 16:] = Ds.reshape(KCH, P).T

    # per-core exercise sort by u (descending) + per-chunk exact-C prefix
    perms = []
    Ts = np.full(KCH, M, dtype=np.int64)
    vlo = vs.reshape(KCH, P).min(axis=1)                       # chunk min v
    for cidx in range(NCORES):
        uc = u[cidx * ROWS:(cidx + 1) * ROWS]
        perm = np.argsort(-uc, kind="stable")
        perms.append(perm)
        us = uc[perm]
        for kk in range(KCH):
            cnt = int((us >= -vlo[kk]).sum())                  # prefix length
            Ts[kk] = min(Ts[kk], cnt)
    Ts = (Ts // 64) * 64                                       # align, pads are B-cols
    Ts = np.minimum(Ts, ROWS // 64 * 64)

    kcp = np.zeros((NKC, D))
    kcp[:kc.shape[0]] = kc
    kcWh = (kcp[order] @ W1).astype(f)                         # [NKC, D]
    kcC = (kcWh * Cs[:, None]).astype(f)                       # C'-folded

    def stat_b(src):
        out = np.zeros((P, NBF * 256), dtype=NP_BF16)
        for kk in range(NBF):
            for t in range(2):
                out[:, kk * 256 + t * P:kk * 256 + (t + 1) * P] = \
                    src[kk * P:(kk + 1) * P, t * P:(t + 1) * P]
        return out

    def stat_8(src):
        s8 = src.astype(NP_FP8)
        out = np.zeros((P, NPAIR * 512), dtype=NP_FP8)
        for pr in range(NPAIR):
            for t in range(2):
                for i in range(2):
                    kk = NBF + 2 * pr + i
                    out[:, pr * 512 + t * 256 + i * P:pr * 512 + t * 256 + (i + 1) * P] = \
                        s8[kk * P:(kk + 1) * P, t * P:(t + 1) * P]
        return out

    kcWhTb = stat_b(kcWh)
    kcCTb = stat_b(kcC)
    kcWhT8 = stat_8(kcWh)
    kcCT8 = stat_8(kcC)
    sCb = np.zeros((P, NBF), dtype=NP_BF16)
    for kk in range(NBF):
        sCb[:, kk] = Cs[kk * P:(kk + 1) * P]
    sC8 = np.zeros((P, NPAIR * 32), dtype=NP_FP8)
    for pr in range(NPAIR):
        for i in range(2):
            kk = NBF + 2 * pr + i
            sC8[:, pr * 32 + i * 16] = Cs[kk * P:(kk + 1) * P]

    EmB = np.zeros((P, 2 * 256), dtype=NP_BF16)
    for cc in range(2):
        for d in range(2):
            EmB[:, cc * 256 + d * P:cc * 256 + (d + 1) * P] = \
                E_[cc * P:(cc + 1) * P, d * P:(d + 1) * P]
    rd_w = np.asarray(rd_w, dtype=np.float64)
    rdwB = np.zeros((P, 4 * 256), dtype=NP_BF16)
    for dd in range(4):
        for oo in range(2):
            rdwB[:, dd * 256 + oo * P:dd * 256 + (oo + 1) * P] = \
                rd_w[oo * P:(oo + 1) * P, dd * P:(dd + 1) * P].T

    shared = {"kcWhT8": kcWhT8, "kcCT8": kcCT8, "sC8": sC8,
              "kcWhTb": kcWhTb, "kcCTb": kcCTb, "sCb": sCb,
              "EmB": EmB, "rdwB": rdwB, "scal": scal}
    maps = []
    for cidx in range(NCORES):
        sl = slice(cidx * ROWS, (cidx + 1) * ROWS)
        perm = perms[cidx]
        rowB_c = np.zeros((1, M), dtype=NP_BF16)
        rowB_c[0, :ROWS] = Brow[sl][perm]
        rowB_c[0, ROWS:] = np.float32(SCALE * np.exp(-c))
        exTb_c = np.zeros((P, 2 * M), dtype=NP_BF16)
        exv = ex[sl].astype(f)[perm]                           # [ROWS, 256]
        exTb_c[:, :ROWS] = exv[:, :P].T
        exTb_c[:, M:M + ROWS] = exv[:, P:].T
        # adj: sorted kc cols, sorted-exercise rows, transpose, chunk
        As = np.zeros((M, NKC), dtype=f)
        real = order < adj.shape[1]
        As[:ROWS, real] = np.asarray(adj[sl], dtype=f)[perm][:, order[real]]
        At = As.T.reshape(KCH, P, M)                           # [kk, p, i]
        m_c = {"rowB": rowB_c, "exTb": exTb_c, **shared}
        for g in range(4):
            ag = np.zeros((P, 4 * M), dtype=NP_FP8)
            for o in range(4):
                ag[:, o * M:(o + 1) * M] = At[g * 4 + o]
            m_c[f"adjg{g}"] = ag
        maps.append(m_c)
    return maps, np.asarray(rd_b, dtype=np.float64), tuple(int(t) for t in Ts), perms


def kernel(exercise_h, kc_h, adj, W1, E, a, rd_w, rd_b):
    maps, rdb, Ts, perms = _prep(exercise_h, kc_h, adj, W1, E, a, rd_w, rd_b)
    nc = _get_program(Ts)
    res = run_bass_kernel_spmd(nc, maps, list(range(NCORES))).results
    out = np.empty((N_E, D), dtype=np.float32)
    for cidx in range(NCORES):
        outBv = np.asarray(res[cidx]["outB"]).astype(np.float64)
        s = np.asarray(res[cidx]["srow"]).astype(np.float64)[0, :ROWS]
        A = outBv.reshape(P, M, 2)
        raw = np.concatenate([A[:, :ROWS, 0].T, A[:, :ROWS, 1].T], axis=1)
        o = raw / s[:, None] + rdb[None, :]
        o = np.where(o > 0, o, np.expm1(np.minimum(o, 0)))
        inv = np.empty(ROWS, dtype=np.int64)
        inv[perms[cidx]] = np.arange(ROWS)
        out[cidx * ROWS:(cidx + 1) * ROWS] = o[inv].astype(np.float32)
    return out


# revision 16
# speedup vs baseline: 1.2595x; 1.2595x over previous
"""GAT-style graph encoder on 8 trn2 NeuronCores — v3.

Reference (per exercise i over kc nodes j):
    kc_Wh = kc_h @ W1; ex_Wh = ex_h @ W1
    e[i,j] = leaky_relu(u_i + v_j, 0.2),  u = ex_Wh@a1, v = kc_Wh@a2
    att = softmax(where(adj>0, e, -9e15), axis=1)
    new_kc = att @ kc_Wh; ex_Eh = ex_h @ E
    out = elu(concat([new_kc, new_kc*ex_Eh]) @ rd_w.T + rd_b)

Strategy (row-shard exercises over 8 cores, 1250 rows -> padded 1280):
The pre-activation logit is separable (u_i + v_j), so with the softmax shift
r_i = u_i + c (softmax is invariant to any per-row scale) the masked exp
factors into rank-1 products:
    p[j,i] = adj * max(C'_j, D_j * B'_i),  C' = e^{v-c}, D = e^{0.2 v},
    B' = e^{-0.8 u - c}    (all host-computed rows; exact algebra).
kc nodes are host-sorted by v (descending), exercises are host-sorted by u
(descending, per core).  Then for each kc chunk there is a column prefix
t_kk = #{i : u_i >= -min_j v_j} where the positive branch wins for EVERY
(j,i) pair, i.e. p = adj * C'_j exactly.  For that prefix the aggregation is
a plain matmul with adj itself as the moving tensor and kcWh*C' folded into
the stationary - no elementwise work at all.  Only the column suffix needs
the two elementwise passes (a 4x DVE tensor_scalar for q and a mask multiply
split across DVE/Pool).  The 4 top (high-v) chunks aggregate in bf16; the 12
tail chunks use fp8e4 DoubleRow matmuls (2 k-tiles/instr at 0.5 cyc/row).
Readout runs in bf16.  The per-row softmax division, +rd_b and elu are
applied on the host during unshard (per-row scalar epilogue).
"""

import ml_dtypes
import numpy as np

import concourse.bacc as bacc
import concourse.bass as bass
import concourse.mybir as mybir
from concourse.alu_op_type import AluOpType
from concourse.bass_utils import run_bass_kernel_spmd
from concourse.tile import TileContext

F32 = mybir.dt.float32
BF16 = mybir.dt.bfloat16
FP8 = mybir.dt.float8e4
DR = mybir.MatmulPerfMode.DoubleRow

P = 128
D = 256
NKC = 2048
KCH = 16                    # kc chunks
NBF = 4                     # leading (high-v) chunks aggregated in bf16
NPAIR = (KCH - NBF) // 2    # fp8 DoubleRow chunk pairs
M = 1280                    # padded exercise rows per core
MBS = (512, 512, 256)
MOFF = (0, 512, 1024)
NCORES = 8
ROWS = 1250
N_E = 10000
SCALE = 128.0               # fp8 range scale folded into B'/C' (cancels in n/s)
# tail chunks whose suffix mask multiply runs on DVE (fp8 out, 1x) vs Pool
DVE_MASK = frozenset((5, 7, 9, 11, 13, 15))

NP_BF16 = ml_dtypes.bfloat16
NP_FP8 = ml_dtypes.float8_e4m3


def _build(Ts):
    """Ts: per-chunk column counts (multiple of 64) where p = adj*C' exactly."""
    nc = bacc.Bacc("TRN2", target_bir_lowering=False, debug=False,
                   num_devices=NCORES)
    adjg = [nc.declare_dram_parameter(f"adjg{g}", [P, 4 * M],
                                      FP8, isOutput=False) for g in range(4)]
    exTb = nc.declare_dram_parameter("exTb", [P, 2 * M], BF16, isOutput=False)
    kcWhT8 = nc.declare_dram_parameter("kcWhT8", [P, NPAIR * 512], FP8,
                                       isOutput=False)
    kcCT8 = nc.declare_dram_parameter("kcCT8", [P, NPAIR * 512], FP8,
                                      isOutput=False)
    sC8 = nc.declare_dram_parameter("sC8", [P, NPAIR * 32], FP8,
                                    isOutput=False)
    kcWhTb = nc.declare_dram_parameter("kcWhTb", [P, NBF * 256], BF16,
                                       isOutput=False)
    kcCTb = nc.declare_dram_parameter("kcCTb", [P, NBF * 256], BF16,
                                      isOutput=False)
    sCb = nc.declare_dram_parameter("sCb", [P, NBF], BF16, isOutput=False)
    EmB = nc.declare_dram_parameter("EmB", [P, 2 * 256], BF16, isOutput=False)
    rdwB = nc.declare_dram_parameter("rdwB", [P, 4 * 256], BF16,
                                     isOutput=False)
    rowB = nc.declare_dram_parameter("rowB", [1, M], BF16, isOutput=False)
    scal = nc.declare_dram_parameter("scal", [P, 32], F32, isOutput=False)
    outB = nc.declare_dram_parameter("outB", [P, 2 * M], BF16, isOutput=True)
    srow = nc.declare_dram_parameter("srow", [1, M], F32, isOutput=True)

    Tpair = [min(Ts[NBF + 2 * pr], Ts[NBF + 2 * pr + 1])
             for pr in range(NPAIR)]

    with TileContext(nc) as tc:
        with tc.tile_pool(name="const", bufs=1) as cpool, \
             tc.tile_pool(name="acc_ps", bufs=2, space="PSUM") as apool, \
             tc.tile_pool(name="out_ps", bufs=2, space="PSUM") as opool, \
             tc.tile_pool(name="mwork", bufs=4) as mpool, \
             tc.tile_pool(name="post", bufs=2) as qpool:
            # ---- const loads: q-gating rows, then aggregation stationaries
            # and adj groups (unblock PE A-matmuls early), then post-stage data
            scal_sb = cpool.tile([P, 32], F32, tag="scal")
            nc.sync.dma_start(out=scal_sb[:], in_=scal[:, :])
            rowB_sb = cpool.tile([1, M], BF16, tag="rowB")
            nc.sync.dma_start(out=rowB_sb[:], in_=rowB[:, :])
            kcCb_sb = cpool.tile([P, NBF * 256], BF16, tag="kcCTb")
            nc.sync.dma_start(out=kcCb_sb[:], in_=kcCTb[:, :])
            kcb_sb = cpool.tile([P, NBF * 256], BF16, tag="kcWhTb")
            nc.sync.dma_start(out=kcb_sb[:], in_=kcWhTb[:, :])
            sCb_sb = cpool.tile([P, NBF], BF16, tag="sCb")
            nc.sync.dma_start(out=sCb_sb[:], in_=sCb[:, :])
            adj_sb = []
            for g in range(4):
                t = cpool.tile([P, 4, M], FP8, tag=f"adjg{g}", name=f"adjg{g}")
                nc.sync.dma_start(out=t[:], in_=adjg[g][:, :])
                adj_sb.append(t)
                if g == 0:
                    # 4-D: [p, (pair,target), ktile=2, m] for DoubleRow APs
                    kc8_sb = cpool.tile([P, NPAIR * 2, 2, P], FP8, tag="kcWhT8")
                    nc.sync.dma_start(out=kc8_sb[:], in_=kcWhT8[:, :])
                    kcC8_sb = cpool.tile([P, NPAIR * 2, 2, P], FP8, tag="kcCT8")
                    nc.sync.dma_start(out=kcC8_sb[:], in_=kcCT8[:, :])
                    sC8_sb = cpool.tile([P, NPAIR, 2, 16], FP8, tag="sC8")
                    nc.sync.dma_start(out=sC8_sb[:], in_=sC8[:, :])
                if g == 1:
                    exT_sb = cpool.tile([P, 2 * M], BF16, tag="exTb")
                    nc.sync.dma_start(out=exT_sb[:], in_=exTb[:, :])
                    EmB_sb = cpool.tile([P, 2 * 256], BF16, tag="EmB")
                    nc.sync.dma_start(out=EmB_sb[:], in_=EmB[:, :])
                if g == 2:
                    rdw_sb = cpool.tile([P, 4 * 256], BF16, tag="rdwB")
                    nc.sync.dma_start(out=rdw_sb[:], in_=rdwB[:, :])

            ones1b = cpool.tile([1, P], BF16, tag="ones1b")
            nc.vector.memset(ones1b[:], 1.0)
            onesb = cpool.tile([P, 1], BF16, tag="onesb")
            nc.vector.memset(onesb[:], 1.0)
            ones8 = cpool.tile([P, 2, 16], FP8, tag="ones8")
            nc.vector.memset(ones8[:], 1.0)
            zerob = cpool.tile([P, P], BF16, tag="zerob")
            nc.vector.memset(zerob[:], 0.0)

            Bb = cpool.tile([P, M], BF16, tag="Bb")          # B' broadcast
            exEhT = [cpool.tile([P, M], BF16, tag=f"exEhT{d}", name=f"exEhT{d}")
                     for d in (0, 1)]
            outB_sb = cpool.tile([P, 2 * M], BF16, tag="outB_sb")
            srow_sb = cpool.tile([1, M], F32, tag="srow_sb")

            # ---- setup: B' broadcast + ex_Eh (psum shared with readout pool)
            for b in range(3):
                ms = slice(MOFF[b], MOFF[b] + MBS[b])
                ps = opool.tile([P, MBS[b]], F32, tag="raw", name=f"bb_ps{b}")
                nc.tensor.matmul(ps[:], ones1b[:], rowB_sb[:, ms],
                                 start=True, stop=True)
                nc.scalar.copy(Bb[:, ms], ps[:])
            for d in range(2):
                for b in range(3):
                    ms = slice(MOFF[b], MOFF[b] + MBS[b])
                    ps = opool.tile([P, MBS[b]], F32, tag="raw",
                                    name=f"eh_ps{d}_{b}")
                    for c in range(2):
                        nc.tensor.matmul(
                            ps[:],
                            EmB_sb[:, c * 256 + d * P:c * 256 + (d + 1) * P],
                            exT_sb[:, c * M + MOFF[b]:c * M + MOFF[b] + MBS[b]],
                            start=(c == 0), stop=(c == 1))
                    nc.scalar.copy(exEhT[d][:, ms], ps[:])

            # ---- suffix-only masked-exp (cols >= T of each chunk)
            def adjsl(kk, lo, hi):
                g, o = divmod(kk, 4)
                return adj_sb[g][:, o, lo:hi]

            def q_of(kk, t0):
                q = mpool.tile([P, M], BF16, tag="q", bufs=4,
                               name=f"q{kk}")
                # (B'_i * D_j) max C'_j : whole unmasked exp in one 4x op
                nc.vector.tensor_scalar(
                    q[:, t0:], Bb[:, t0:], scal_sb[:, 16 + kk:17 + kk],
                    scal_sb[:, kk:kk + 1], AluOpType.mult, AluOpType.max)
                return q

            ptmb = []
            for kk in range(NBF):
                t0 = Ts[kk]
                if t0 >= M:
                    ptmb.append(None)
                    continue
                q = q_of(kk, t0)
                t = mpool.tile([P, M], BF16, tag="ptmb", bufs=NBF,
                               name=f"ptmb{kk}")
                nc.vector.tensor_mul(t[:, t0:], q[:, t0:], adjsl(kk, t0, M))
                ptmb.append(t)
            ptm8 = []
            for pr in range(NPAIR):     # pairs already follow adj group order
                t0 = Tpair[pr]
                if t0 >= M:
                    ptm8.append(None)
                    continue
                t8 = mpool.tile([P, 2, M], FP8, tag="ptm8", bufs=NPAIR,
                                name=f"ptm8_{pr}")
                for h in range(2):
                    kk = NBF + 2 * pr + h
                    q = q_of(kk, t0)
                    eng = nc.vector if kk in DVE_MASK else nc.gpsimd
                    eng.tensor_mul(t8[:, h, t0:], q[:, t0:], adjsl(kk, t0, M))
                ptm8.append(t8)

            # ---- per m-block aggregation + readout
            for b in range(3):
                mb = MBS[b]
                ms = slice(MOFF[b], MOFF[b] + mb)
                n0 = apool.tile([P, mb], F32, tag="n0")
                n1 = apool.tile([P, mb], F32, tag="n1")
                sS = apool.tile([1, mb], F32, tag="sS")

                # start=True zeroes the whole 2KB psum bank, so the group
                # opener must be a full-width matmul emitted first: prefer a
                # chunk whose exact-C prefix covers the block (adj-only dep,
                # lets PE start without waiting for masks), else open with a
                # zero-stationary matmul on an always-ready moving tile.
                starter = None
                for kk in range(NBF):
                    if Ts[kk] - MOFF[b] >= mb:
                        starter = kk
                        break
                if starter is None:
                    nc.tensor.matmul(n0[:], zerob[:], Bb[:, ms],
                                     start=True, stop=False,
                                     skip_group_check=True)
                    nc.tensor.matmul(n1[:], zerob[:], Bb[:, ms],
                                     start=True, stop=False,
                                     skip_group_check=True)
                    nc.tensor.matmul(sS[:], zerob[:, 0:1], Bb[:, ms],
                                     start=True, stop=False,
                                     skip_group_check=True)
                order = ([starter] + [k for k in range(NBF) if k != starter]
                         if starter is not None else list(range(NBF)))
                for kk in order:
                    aw = min(max(Ts[kk] - MOFF[b], 0), mb)
                    st = (starter is not None and kk == starter)
                    if aw > 0:
                        asl = adjsl(kk, MOFF[b], MOFF[b] + aw)
                        nc.tensor.matmul(
                            n0[:, 0:aw], kcCb_sb[:, kk * 256:kk * 256 + P],
                            asl, start=st, stop=False, skip_group_check=True)
                        nc.tensor.matmul(
                            n1[:, 0:aw], kcCb_sb[:, kk * 256 + P:(kk + 1) * 256],
                            asl, start=st, stop=False, skip_group_check=True)
                        nc.tensor.matmul(
                            sS[:, 0:aw], sCb_sb[:, kk:kk + 1],
                            asl, start=st, stop=False, skip_group_check=True)
                    if aw < mb:
                        pm = ptmb[kk][:, MOFF[b] + aw:MOFF[b] + mb]
                        nc.tensor.matmul(
                            n0[:, aw:mb], kcb_sb[:, kk * 256:kk * 256 + P],
                            pm, start=st, stop=False, skip_group_check=True)
                        nc.tensor.matmul(
                            n1[:, aw:mb], kcb_sb[:, kk * 256 + P:(kk + 1) * 256],
                            pm, start=st, stop=False, skip_group_check=True)
                        nc.tensor.matmul(
                            sS[:, aw:mb], onesb[:],
                            pm, start=st, stop=False, skip_group_check=True)
                # fp8 DoubleRow pairs
                for pr in range(NPAIR):
                    aw = min(max(Tpair[pr] - MOFF[b], 0), mb)
                    g, o = divmod(NBF + 2 * pr, 4)
                    sp = (pr == NPAIR - 1)
                    if aw > 0:
                        adjpair = adj_sb[g][:, o:o + 2, MOFF[b]:MOFF[b] + aw]
                        nc.tensor.matmul(
                            n0[:, 0:aw], kcC8_sb[:, 2 * pr, :, :], adjpair,
                            start=False, stop=sp and aw >= mb,
                            perf_mode=DR, skip_group_check=True)
                        nc.tensor.matmul(
                            n1[:, 0:aw], kcC8_sb[:, 2 * pr + 1, :, :], adjpair,
                            start=False, stop=sp and aw >= mb,
                            perf_mode=DR, skip_group_check=True)
                        nc.tensor.matmul(
                            sS[:, 0:aw], sC8_sb[:, pr, :, 0:1], adjpair,
                            start=False, stop=sp and aw >= mb,
                            perf_mode=DR, skip_group_check=True)
                    if aw < mb:
                        pm = ptm8[pr][:, :, MOFF[b] + aw:MOFF[b] + mb]
                        nc.tensor.matmul(
                            n0[:, aw:mb], kc8_sb[:, 2 * pr, :, :], pm,
                            start=False, stop=sp, perf_mode=DR,
                            skip_group_check=True)
                        nc.tensor.matmul(
                            n1[:, aw:mb], kc8_sb[:, 2 * pr + 1, :, :], pm,
                            start=False, stop=sp, perf_mode=DR,
                            skip_group_check=True)
                        nc.tensor.matmul(
                            sS[:, aw:mb], ones8[:, :, 0:1], pm,
                            start=False, stop=sp, perf_mode=DR,
                            skip_group_check=True)

                # ---- post: features, readout, stage out
                nc.vector.tensor_copy(srow_sb[:, ms], sS[:])
                ncf = []
                for t in range(2):
                    nt = qpool.tile([P, mb], BF16, tag=f"nc{t}", name=f"nc{t}")
                    nc.scalar.copy(nt[:], (n0 if t == 0 else n1)[:])
                    ncf.append(nt)
                tf = []
                for t in range(2):
                    tt = qpool.tile([P, mb], BF16, tag=f"t{t}", name=f"tt{t}")
                    nc.vector.tensor_mul(tt[:], ncf[t][:], exEhT[t][:, ms])
                    tf.append(tt)
                feats = [ncf[0], ncf[1], tf[0], tf[1]]
                for oo in range(2):
                    raw = opool.tile([P, mb], F32, tag="raw")
                    for dd in range(4):
                        nc.tensor.matmul(
                            raw[:], rdw_sb[:, dd * 256 + oo * P:dd * 256 + (oo + 1) * P],
                            feats[dd][:], start=(dd == 0), stop=(dd == 3))
                    # stage to outB interleaved (col 2i+oo) for one DMA/block
                    nc.scalar.copy(
                        outB_sb[:, 2 * MOFF[b] + oo:2 * (MOFF[b] + mb):2],
                        raw[:])
                nc.sync.dma_start(
                    out=outB[:, 2 * MOFF[b]:2 * (MOFF[b] + mb)],
                    in_=outB_sb[:, 2 * MOFF[b]:2 * (MOFF[b] + mb)])
            nc.sync.dma_start(out=srow[:, :], in_=srow_sb[:])
    nc.finalize()
    return nc


_PROGRAMS = {}


def _get_program(Ts):
    key = tuple(Ts)
    if key not in _PROGRAMS:
        _PROGRAMS[key] = _build(key)
    return _PROGRAMS[key]


def _prep(exercise_h, kc_h, adj, W1, E, a, rd_w, rd_b):
    f = np.float32
    ex = np.asarray(exercise_h, dtype=np.float64)
    kc = np.asarray(kc_h, dtype=np.float64)
    W1 = np.asarray(W1, dtype=np.float64)
    E_ = np.asarray(E, dtype=np.float64)
    a = np.asarray(a, dtype=np.float64)
    a1, a2 = a[:D, 0], a[D:, 0]

    u = ex @ (W1 @ a1)                        # [N_E]
    vp = np.full(NKC, -60.0)
    vp[:kc.shape[0]] = kc @ (W1 @ a2)
    order = np.argsort(-vp, kind="stable")
    vs = vp[order]
    vmax = vs[0]
    c = float((np.maximum(u + vmax, 0.2 * (u + vmax)) - u).max())

    Brow = (SCALE * np.exp(-0.8 * u - c)).astype(f)            # [N_E]
    Cs = (SCALE * np.exp(vs - c)).astype(f)                    # [NKC]
    Ds = np.exp(0.2 * vs).astype(f)                            # [NKC]
    scal = np.zeros((P, 32), dtype=f)
    scal[:, :16] = Cs.reshape(KCH, P).T
    scal[:, 16:] = Ds.reshape(KCH, P).T

    # per-core exercise sort by u (descending) + per-chunk exact-C prefix
    perms = []
    Ts = np.full(KCH, M, dtype=np.int64)
    vlo = vs.reshape(KCH, P).min(axis=1)                       # chunk min v
    for cidx in range(NCORES):
        uc = u[cidx * ROWS:(cidx + 1) * ROWS]
        perm = np.argsort(-uc, kind="stable")
        perms.append(perm)
        us = uc[perm]
        for kk in range(KCH):
            cnt = int((us >= -vlo[kk]).sum())                  # prefix length
            Ts[kk] = min(Ts[kk], cnt)
    Ts = (Ts // 64) * 64                                       # align, pads are B-cols
    Ts = np.minimum(Ts, ROWS // 64 * 64)

    kcp = np.zeros((NKC, D))
    kcp[:kc.shape[0]] = kc
    kcWh = (kcp[order] @ W1).astype(f)                         # [NKC, D]
    kcC = (kcWh * Cs[:, None]).astype(f)                       # C'-folded

    def stat_b(src):
        out = np.zeros((P, NBF * 256), dtype=NP_BF16)
        for kk in range(NBF):
            for t in range(2):
                out[:, kk * 256 + t * P:kk * 256 + (t + 1) * P] = \
                    src[kk * P:(kk + 1) * P, t * P:(t + 1) * P]
        return out

    def stat_8(src):
        s8 = src.astype(NP_FP8)
        out = np.zeros((P, NPAIR * 512), dtype=NP_FP8)
        for pr in range(NPAIR):
            for t in range(2):
                for i in range(2):
                    kk = NBF + 2 * pr + i
                    out[:, pr * 512 + t * 256 + i * P:pr * 512 + t * 256 + (i + 1) * P] = \
                        s8[kk * P:(kk + 1) * P, t * P:(t + 1) * P]
        return out

    kcWhTb = stat_b(kcWh)
    kcCTb = stat_b(kcC)
    kcWhT8 = stat_8(kcWh)
    kcCT8 = stat_8(kcC)
    sCb = np.zeros((P, NBF), dtype=NP_BF16)
    for kk in range(NBF):
        sCb[:, kk] = Cs[kk * P:(kk + 1) * P]
    sC8 = np.zeros((P, NPAIR * 32), dtype=NP_FP8)
    for pr in range(NPAIR):
        for i in range(2):
            kk = NBF + 2 * pr + i
            sC8[:, pr * 32 + i * 16] = Cs[kk * P:(kk + 1) * P]

    EmB = np.zeros((P, 2 * 256), dtype=NP_BF16)
    for cc in range(2):
        for d in range(2):
            EmB[:, cc * 256 + d * P:cc * 256 + (d + 1) * P] = \
                E_[cc * P:(cc + 1) * P, d * P:(d + 1) * P]
    rd_w = np.asarray(rd_w, dtype=np.float64)
    rdwB = np.zeros((P, 4 * 256), dtype=NP_BF16)
    for dd in range(4):
        for oo in range(2):
            rdwB[:, dd * 256 + oo * P:dd * 256 + (oo + 1) * P] = \
                rd_w[oo * P:(oo + 1) * P, dd * P:(dd + 1) * P].T

    shared = {"kcWhT8": kcWhT8, "kcCT8": kcCT8, "sC8": sC8,
              "kcWhTb": kcWhTb, "kcCTb": kcCTb, "sCb": sCb,
              "EmB": EmB, "rdwB": rdwB, "scal": scal}
    maps = []
    for cidx in range(NCORES):
        sl = slice(cidx * ROWS, (cidx + 1) * ROWS)
        perm = perms[cidx]
        rowB_c = np.zeros((1, M), dtype=NP_BF16)
        rowB_c[0, :ROWS] = Brow[sl][perm]
        rowB_c[0, ROWS:] = np.float32(SCALE * np.exp(-c))
        exTb_c = np.zeros((P, 2 * M), dtype=NP_BF16)
        exv = ex[sl].astype(f)[perm]                           # [ROWS, 256]
        exTb_c[:, :ROWS] = exv[:, :P].T
        exTb_c[:, M:M + ROWS] = exv[:, P:].T
        # adj: sorted kc cols, sorted-exercise rows, transpose, chunk
        As = np.zeros((M, NKC), dtype=f)
        real = order < adj.shape[1]
        As[:ROWS, real] = np.asarray(adj[sl], dtype=f)[perm][:, order[real]]
        At = As.T.reshape(KCH, P, M)                           # [kk, p, i]
        m_c = {"rowB": rowB_c, "exTb": exTb_c, **shared}
        for g in range(4):
            ag = np.zeros((P, 4 * M), dtype=NP_FP8)
            for o in range(4):
                ag[:, o * M:(o + 1) * M] = At[g * 4 + o]
            m_c[f"adjg{g}"] = ag
        maps.append(m_c)
    return maps, np.asarray(rd_b, dtype=np.float64), tuple(int(t) for t in Ts), perms


def kernel(exercise_h, kc_h, adj, W1, E, a, rd_w, rd_b):
    maps, rdb, Ts, perms = _prep(exercise_h, kc_h, adj, W1, E, a, rd_w, rd_b)
    nc = _get_program(Ts)
    res = run_bass_kernel_spmd(nc, maps, list(range(NCORES))).results
    out = np.empty((N_E, D), dtype=np.float32)
    for cidx in range(NCORES):
        outBv = np.asarray(res[cidx]["outB"]).astype(np.float64)
        s = np.asarray(res[cidx]["srow"]).astype(np.float64)[0, :ROWS]
        A = outBv.reshape(P, M, 2)
        raw = np.concatenate([A[:, :ROWS, 0].T, A[:, :ROWS, 1].T], axis=1)
        o = raw / s[:, None] + rdb[None, :]
        o = np.where(o > 0, o, np.expm1(np.minimum(o, 0)))
        inv = np.empty(ROWS, dtype=np.int64)
        inv[perms[cidx]] = np.arange(ROWS)
        out[cidx * ROWS:(cidx + 1) * ROWS] = o[inv].astype(np.float32)
    return out


# revision 17
# speedup vs baseline: 1.2685x; 1.0072x over previous
"""GAT-style graph encoder on 8 trn2 NeuronCores — v3.

Reference (per exercise i over kc nodes j):
    kc_Wh = kc_h @ W1; ex_Wh = ex_h @ W1
    e[i,j] = leaky_relu(u_i + v_j, 0.2),  u = ex_Wh@a1, v = kc_Wh@a2
    att = softmax(where(adj>0, e, -9e15), axis=1)
    new_kc = att @ kc_Wh; ex_Eh = ex_h @ E
    out = elu(concat([new_kc, new_kc*ex_Eh]) @ rd_w.T + rd_b)

Strategy (row-shard exercises over 8 cores, 1250 rows -> padded 1280):
The pre-activation logit is separable (u_i + v_j), so with the softmax shift
r_i = u_i + c (softmax is invariant to any per-row scale) the masked exp
factors into rank-1 products:
    p[j,i] = adj * max(C'_j, D_j * B'_i),  C' = e^{v-c}, D = e^{0.2 v},
    B' = e^{-0.8 u - c}    (all host-computed rows; exact algebra).
kc nodes are host-sorted by v (descending), exercises are host-sorted by u
(descending, per core).  Then for each kc chunk there is a column prefix
t_kk = #{i : u_i >= -min_j v_j} where the positive branch wins for EVERY
(j,i) pair, i.e. p = adj * C'_j exactly.  For that prefix the aggregation is
a plain matmul with adj itself as the moving tensor and kcWh*C' folded into
the stationary - no elementwise work at all.  Only the column suffix needs
the two elementwise passes (a 4x DVE tensor_scalar for q and a mask multiply
split across DVE/Pool).  The 4 top (high-v) chunks aggregate in bf16; the 12
tail chunks use fp8e4 DoubleRow matmuls (2 k-tiles/instr at 0.5 cyc/row).
Readout runs in bf16.  The per-row softmax division, +rd_b and elu are
applied on the host during unshard (per-row scalar epilogue).
"""

import ml_dtypes
import numpy as np

import concourse.bacc as bacc
import concourse.bass as bass
import concourse.mybir as mybir
from concourse.alu_op_type import AluOpType
from concourse.bass_utils import run_bass_kernel_spmd
from concourse.tile import TileContext

F32 = mybir.dt.float32
BF16 = mybir.dt.bfloat16
FP8 = mybir.dt.float8e4
DR = mybir.MatmulPerfMode.DoubleRow

P = 128
D = 256
NKC = 2048
KCH = 16                    # kc chunks
NBF = 4                     # leading (high-v) chunks aggregated in bf16
NPAIR = (KCH - NBF) // 2    # fp8 DoubleRow chunk pairs
M = 1280                    # padded exercise rows per core
MBS = (512, 512, 256)
MOFF = (0, 512, 1024)
NCORES = 8
ROWS = 1250
N_E = 10000
SCALE = 128.0               # fp8 range scale folded into B'/C' (cancels in n/s)
# tail chunks whose suffix mask multiply runs on DVE (fp8 out, 1x) vs Pool
DVE_MASK = frozenset((5, 7, 9, 11, 13, 15))

NP_BF16 = ml_dtypes.bfloat16
NP_FP8 = ml_dtypes.float8_e4m3


def _build(Ts):
    """Ts: per-chunk column counts (multiple of 64) where p = adj*C' exactly."""
    nc = bacc.Bacc("TRN2", target_bir_lowering=False, debug=False,
                   num_devices=NCORES)
    adjg = [nc.declare_dram_parameter(f"adjg{g}", [P, 2 * M],
                                      FP8, isOutput=False) for g in range(8)]
    exTb = nc.declare_dram_parameter("exTb", [P, 2 * M], BF16, isOutput=False)
    kcWhT8 = nc.declare_dram_parameter("kcWhT8", [P, NPAIR * 512], FP8,
                                       isOutput=False)
    kcCT8 = nc.declare_dram_parameter("kcCT8", [P, NPAIR * 512], FP8,
                                      isOutput=False)
    sC8 = nc.declare_dram_parameter("sC8", [P, NPAIR * 32], FP8,
                                    isOutput=False)
    kcWhTb = nc.declare_dram_parameter("kcWhTb", [P, NBF * 256], BF16,
                                       isOutput=False)
    kcCTb = nc.declare_dram_parameter("kcCTb", [P, NBF * 256], BF16,
                                      isOutput=False)
    sCb = nc.declare_dram_parameter("sCb", [P, NBF], BF16, isOutput=False)
    EmB = nc.declare_dram_parameter("EmB", [P, 2 * 256], BF16, isOutput=False)
    rdwB = nc.declare_dram_parameter("rdwB", [P, 4 * 256], BF16,
                                     isOutput=False)
    rowB = nc.declare_dram_parameter("rowB", [1, M], BF16, isOutput=False)
    scal = nc.declare_dram_parameter("scal", [P, 32], F32, isOutput=False)
    outB = nc.declare_dram_parameter("outB", [P, 2 * M], BF16, isOutput=True)
    srow = nc.declare_dram_parameter("srow", [1, M], F32, isOutput=True)

    Tpair = [min(Ts[NBF + 2 * pr], Ts[NBF + 2 * pr + 1])
             for pr in range(NPAIR)]

    with TileContext(nc) as tc:
        with tc.tile_pool(name="const", bufs=1) as cpool, \
             tc.tile_pool(name="acc_ps", bufs=2, space="PSUM") as apool, \
             tc.tile_pool(name="out_ps", bufs=2, space="PSUM") as opool, \
             tc.tile_pool(name="mwork", bufs=4) as mpool, \
             tc.tile_pool(name="post", bufs=2) as qpool:
            # ---- const loads: q-gating rows, then aggregation stationaries
            # and adj groups (unblock PE A-matmuls early), then post-stage data
            scal_sb = cpool.tile([P, 32], F32, tag="scal")
            nc.sync.dma_start(out=scal_sb[:], in_=scal[:, :])
            rowB_sb = cpool.tile([1, M], BF16, tag="rowB")
            nc.sync.dma_start(out=rowB_sb[:], in_=rowB[:, :])
            kcCb_sb = cpool.tile([P, NBF * 256], BF16, tag="kcCTb")
            nc.sync.dma_start(out=kcCb_sb[:], in_=kcCTb[:, :])
            kcb_sb = cpool.tile([P, NBF * 256], BF16, tag="kcWhTb")
            nc.sync.dma_start(out=kcb_sb[:], in_=kcWhTb[:, :])
            sCb_sb = cpool.tile([P, NBF], BF16, tag="sCb")
            nc.sync.dma_start(out=sCb_sb[:], in_=sCb[:, :])
            # adj pair-tiles loaded big-mask-suffix first, interleaved with
            # stationaries/post-stage consts; index = pair position (kk//2)
            ADJ_ORDER = (7, 0, 6, 1, 5, 2, 4, 3)
            adj_sb = [None] * 8
            for step, g in enumerate(ADJ_ORDER):
                t = cpool.tile([P, 2, M], FP8, tag=f"adjg{g}", name=f"adjg{g}")
                nc.sync.dma_start(out=t[:], in_=adjg[g][:, :])
                adj_sb[g] = t
                if step == 0:
                    # 4-D: [p, (pair,target), ktile=2, m] for DoubleRow APs
                    kc8_sb = cpool.tile([P, NPAIR * 2, 2, P], FP8, tag="kcWhT8")
                    nc.sync.dma_start(out=kc8_sb[:], in_=kcWhT8[:, :])
                    kcC8_sb = cpool.tile([P, NPAIR * 2, 2, P], FP8, tag="kcCT8")
                    nc.sync.dma_start(out=kcC8_sb[:], in_=kcCT8[:, :])
                    sC8_sb = cpool.tile([P, NPAIR, 2, 16], FP8, tag="sC8")
                    nc.sync.dma_start(out=sC8_sb[:], in_=sC8[:, :])
                if step == 2:
                    exT_sb = cpool.tile([P, 2 * M], BF16, tag="exTb")
                    nc.sync.dma_start(out=exT_sb[:], in_=exTb[:, :])
                    EmB_sb = cpool.tile([P, 2 * 256], BF16, tag="EmB")
                    nc.sync.dma_start(out=EmB_sb[:], in_=EmB[:, :])
                if step == 4:
                    rdw_sb = cpool.tile([P, 4 * 256], BF16, tag="rdwB")
                    nc.sync.dma_start(out=rdw_sb[:], in_=rdwB[:, :])

            ones1b = cpool.tile([1, P], BF16, tag="ones1b")
            nc.vector.memset(ones1b[:], 1.0)
            onesb = cpool.tile([P, 1], BF16, tag="onesb")
            nc.vector.memset(onesb[:], 1.0)
            ones8 = cpool.tile([P, 2, 16], FP8, tag="ones8")
            nc.vector.memset(ones8[:], 1.0)
            zerob = cpool.tile([P, P], BF16, tag="zerob")
            nc.vector.memset(zerob[:], 0.0)

            Bb = cpool.tile([P, M], BF16, tag="Bb")          # B' broadcast
            exEhT = [cpool.tile([P, M], BF16, tag=f"exEhT{d}", name=f"exEhT{d}")
                     for d in (0, 1)]
            outB_sb = cpool.tile([P, 2 * M], BF16, tag="outB_sb")
            srow_sb = cpool.tile([1, M], F32, tag="srow_sb")

            # ---- setup: B' broadcast + ex_Eh (psum shared with readout pool)
            for b in range(3):
                ms = slice(MOFF[b], MOFF[b] + MBS[b])
                ps = opool.tile([P, MBS[b]], F32, tag="raw", name=f"bb_ps{b}")
                nc.tensor.matmul(ps[:], ones1b[:], rowB_sb[:, ms],
                                 start=True, stop=True)
                nc.scalar.copy(Bb[:, ms], ps[:])
            for d in range(2):
                for b in range(3):
                    ms = slice(MOFF[b], MOFF[b] + MBS[b])
                    ps = opool.tile([P, MBS[b]], F32, tag="raw",
                                    name=f"eh_ps{d}_{b}")
                    for c in range(2):
                        nc.tensor.matmul(
                            ps[:],
                            EmB_sb[:, c * 256 + d * P:c * 256 + (d + 1) * P],
                            exT_sb[:, c * M + MOFF[b]:c * M + MOFF[b] + MBS[b]],
                            start=(c == 0), stop=(c == 1))
                    nc.scalar.copy(exEhT[d][:, ms], ps[:])

            # ---- suffix-only masked-exp (cols >= T of each chunk)
            def adjsl(kk, lo, hi):
                g, o = divmod(kk, 2)
                return adj_sb[g][:, o, lo:hi]

            def q_of(kk, t0):
                q = mpool.tile([P, M], BF16, tag="q", bufs=4,
                               name=f"q{kk}")
                # (B'_i * D_j) max C'_j : whole unmasked exp in one 4x op
                nc.vector.tensor_scalar(
                    q[:, t0:], Bb[:, t0:], scal_sb[:, 16 + kk:17 + kk],
                    scal_sb[:, kk:kk + 1], AluOpType.mult, AluOpType.max)
                return q

            # q rows (full-width, cheap 4x DVE) in mask-priority order;
            # mask multiplies are emitted per m-block (block-major) so early
            # blocks close their accumulations before the last masks finish
            MASK_ORDER = (14, 15, 0, 1, 12, 13, 2, 3, 10, 11, 4, 5, 8, 9, 6, 7)
            qs, ptmb, ptm8 = {}, [None] * NBF, [None] * NPAIR
            for kk in MASK_ORDER:
                t0 = Ts[kk] if kk < NBF else Tpair[(kk - NBF) // 2]
                if t0 >= M:
                    continue
                qs[kk] = q_of(kk, t0)
                if kk < NBF:
                    ptmb[kk] = mpool.tile([P, M], BF16, tag="ptmb", bufs=NBF,
                                          name=f"ptmb{kk}")
                else:
                    pr = (kk - NBF) // 2
                    if ptm8[pr] is None:
                        ptm8[pr] = mpool.tile([P, 2, M], FP8, tag="ptm8",
                                              bufs=NPAIR, name=f"ptm8_{pr}")
            for b in range(3):
                for kk in MASK_ORDER:
                    t0 = Ts[kk] if kk < NBF else Tpair[(kk - NBF) // 2]
                    lo = max(t0, MOFF[b])
                    hi = MOFF[b] + MBS[b]
                    if lo >= hi:
                        continue
                    if kk < NBF:
                        nc.vector.tensor_mul(ptmb[kk][:, lo:hi],
                                             qs[kk][:, lo:hi],
                                             adjsl(kk, lo, hi))
                    else:
                        pr, h = divmod(kk - NBF, 2)
                        eng = nc.vector if kk in DVE_MASK else nc.gpsimd
                        eng.tensor_mul(ptm8[pr][:, h, lo:hi],
                                       qs[kk][:, lo:hi], adjsl(kk, lo, hi))

            # ---- per m-block aggregation + readout
            for b in range(3):
                mb = MBS[b]
                ms = slice(MOFF[b], MOFF[b] + mb)
                n0 = apool.tile([P, mb], F32, tag="n0")
                n1 = apool.tile([P, mb], F32, tag="n1")
                sS = apool.tile([1, mb], F32, tag="sS")

                # start=True zeroes the whole 2KB psum bank, so the group
                # opener must be a full-width matmul emitted first: prefer a
                # chunk whose exact-C prefix covers the block (adj-only dep,
                # lets PE start without waiting for masks), else open with a
                # zero-stationary matmul on an always-ready moving tile.
                starter = None
                for kk in range(NBF):
                    if Ts[kk] - MOFF[b] >= mb:
                        starter = kk
                        break
                if starter is None:
                    nc.tensor.matmul(n0[:], zerob[:], Bb[:, ms],
                                     start=True, stop=False,
                                     skip_group_check=True)
                    nc.tensor.matmul(n1[:], zerob[:], Bb[:, ms],
                                     start=True, stop=False,
                                     skip_group_check=True)
                    nc.tensor.matmul(sS[:], zerob[:, 0:1], Bb[:, ms],
                                     start=True, stop=False,
                                     skip_group_check=True)
                order = ([starter] + [k for k in range(NBF) if k != starter]
                         if starter is not None else list(range(NBF)))
                for kk in order:
                    aw = min(max(Ts[kk] - MOFF[b], 0), mb)
                    st = (starter is not None and kk == starter)
                    if aw > 0:
                        asl = adjsl(kk, MOFF[b], MOFF[b] + aw)
                        nc.tensor.matmul(
                            n0[:, 0:aw], kcCb_sb[:, kk * 256:kk * 256 + P],
                            asl, start=st, stop=False, skip_group_check=True)
                        nc.tensor.matmul(
                            n1[:, 0:aw], kcCb_sb[:, kk * 256 + P:(kk + 1) * 256],
                            asl, start=st, stop=False, skip_group_check=True)
                        nc.tensor.matmul(
                            sS[:, 0:aw], sCb_sb[:, kk:kk + 1],
                            asl, start=st, stop=False, skip_group_check=True)
                    if aw < mb:
                        pm = ptmb[kk][:, MOFF[b] + aw:MOFF[b] + mb]
                        nc.tensor.matmul(
                            n0[:, aw:mb], kcb_sb[:, kk * 256:kk * 256 + P],
                            pm, start=st, stop=False, skip_group_check=True)
                        nc.tensor.matmul(
                            n1[:, aw:mb], kcb_sb[:, kk * 256 + P:(kk + 1) * 256],
                            pm, start=st, stop=False, skip_group_check=True)
                        nc.tensor.matmul(
                            sS[:, aw:mb], onesb[:],
                            pm, start=st, stop=False, skip_group_check=True)
                # fp8 DoubleRow pairs
                for pr in range(NPAIR):
                    aw = min(max(Tpair[pr] - MOFF[b], 0), mb)
                    sp = (pr == NPAIR - 1)
                    if aw > 0:
                        adjpair = adj_sb[(NBF + 2 * pr) // 2][:, :, MOFF[b]:MOFF[b] + aw]
                        nc.tensor.matmul(
                            n0[:, 0:aw], kcC8_sb[:, 2 * pr, :, :], adjpair,
                            start=False, stop=sp and aw >= mb,
                            perf_mode=DR, skip_group_check=True)
                        nc.tensor.matmul(
                            n1[:, 0:aw], kcC8_sb[:, 2 * pr + 1, :, :], adjpair,
                            start=False, stop=sp and aw >= mb,
                            perf_mode=DR, skip_group_check=True)
                        nc.tensor.matmul(
                            sS[:, 0:aw], sC8_sb[:, pr, :, 0:1], adjpair,
                            start=False, stop=sp and aw >= mb,
                            perf_mode=DR, skip_group_check=True)
                    if aw < mb:
                        pm = ptm8[pr][:, :, MOFF[b] + aw:MOFF[b] + mb]
                        nc.tensor.matmul(
                            n0[:, aw:mb], kc8_sb[:, 2 * pr, :, :], pm,
                            start=False, stop=sp, perf_mode=DR,
                            skip_group_check=True)
                        nc.tensor.matmul(
                            n1[:, aw:mb], kc8_sb[:, 2 * pr + 1, :, :], pm,
                            start=False, stop=sp, perf_mode=DR,
                            skip_group_check=True)
                        nc.tensor.matmul(
                            sS[:, aw:mb], ones8[:, :, 0:1], pm,
                            start=False, stop=sp, perf_mode=DR,
                            skip_group_check=True)

                # ---- post: features, readout, stage out
                nc.vector.tensor_copy(srow_sb[:, ms], sS[:])
                ncf = []
                for t in range(2):
                    nt = qpool.tile([P, mb], BF16, tag=f"nc{t}", name=f"nc{t}")
                    nc.scalar.copy(nt[:], (n0 if t == 0 else n1)[:])
                    ncf.append(nt)
                tf = []
                for t in range(2):
                    tt = qpool.tile([P, mb], BF16, tag=f"t{t}", name=f"tt{t}")
                    nc.vector.tensor_mul(tt[:], ncf[t][:], exEhT[t][:, ms])
                    tf.append(tt)
                feats = [ncf[0], ncf[1], tf[0], tf[1]]
                for oo in range(2):
                    raw = opool.tile([P, mb], F32, tag="raw")
                    for dd in range(4):
                        nc.tensor.matmul(
                            raw[:], rdw_sb[:, dd * 256 + oo * P:dd * 256 + (oo + 1) * P],
                            feats[dd][:], start=(dd == 0), stop=(dd == 3))
                    # stage to outB interleaved (col 2i+oo) for one DMA/block
                    nc.scalar.copy(
                        outB_sb[:, 2 * MOFF[b] + oo:2 * (MOFF[b] + mb):2],
                        raw[:])
                nc.sync.dma_start(
                    out=outB[:, 2 * MOFF[b]:2 * (MOFF[b] + mb)],
                    in_=outB_sb[:, 2 * MOFF[b]:2 * (MOFF[b] + mb)])
            nc.sync.dma_start(out=srow[:, :], in_=srow_sb[:])
    nc.finalize()
    return nc


_PROGRAMS = {}


def _get_program(Ts):
    key = tuple(Ts)
    if key not in _PROGRAMS:
        _PROGRAMS[key] = _build(key)
    return _PROGRAMS[key]


def _prep(exercise_h, kc_h, adj, W1, E, a, rd_w, rd_b):
    f = np.float32
    ex = np.asarray(exercise_h, dtype=np.float64)
    kc = np.asarray(kc_h, dtype=np.float64)
    W1 = np.asarray(W1, dtype=np.float64)
    E_ = np.asarray(E, dtype=np.float64)
    a = np.asarray(a, dtype=np.float64)
    a1, a2 = a[:D, 0], a[D:, 0]

    u = ex @ (W1 @ a1)                        # [N_E]
    vp = np.full(NKC, -60.0)
    vp[:kc.shape[0]] = kc @ (W1 @ a2)
    order = np.argsort(-vp, kind="stable")
    vs = vp[order]
    vmax = vs[0]
    c = float((np.maximum(u + vmax, 0.2 * (u + vmax)) - u).max())

    Brow = (SCALE * np.exp(-0.8 * u - c)).astype(f)            # [N_E]
    Cs = (SCALE * np.exp(vs - c)).astype(f)                    # [NKC]
    Ds = np.exp(0.2 * vs).astype(f)                            # [NKC]
    scal = np.zeros((P, 32), dtype=f)
    scal[:, :16] = Cs.reshape(KCH, P).T
    scal[:, 16:] = Ds.reshape(KCH, P).T

    # per-core exercise sort by u (descending) + per-chunk exact-C prefix
    perms = []
    Ts = np.full(KCH, M, dtype=np.int64)
    vlo = vs.reshape(KCH, P).min(axis=1)                       # chunk min v
    for cidx in range(NCORES):
        uc = u[cidx * ROWS:(cidx + 1) * ROWS]
        perm = np.argsort(-uc, kind="stable")
        perms.append(perm)
        us = uc[perm]
        for kk in range(KCH):
            cnt = int((us >= -vlo[kk]).sum())                  # prefix length
            Ts[kk] = min(Ts[kk], cnt)
    Ts = (Ts // 64) * 64                                       # align, pads are B-cols
    Ts = np.minimum(Ts, ROWS // 64 * 64)

    kcp = np.zeros((NKC, D))
    kcp[:kc.shape[0]] = kc
    kcWh = (kcp[order] @ W1).astype(f)                         # [NKC, D]
    kcC = (kcWh * Cs[:, None]).astype(f)                       # C'-folded

    def stat_b(src):
        out = np.zeros((P, NBF * 256), dtype=NP_BF16)
        for kk in range(NBF):
            for t in range(2):
                out[:, kk * 256 + t * P:kk * 256 + (t + 1) * P] = \
                    src[kk * P:(kk + 1) * P, t * P:(t + 1) * P]
        return out

    def stat_8(src):
        s8 = src.astype(NP_FP8)
        out = np.zeros((P, NPAIR * 512), dtype=NP_FP8)
        for pr in range(NPAIR):
            for t in range(2):
                for i in range(2):
                    kk = NBF + 2 * pr + i
                    out[:, pr * 512 + t * 256 + i * P:pr * 512 + t * 256 + (i + 1) * P] = \
                        s8[kk * P:(kk + 1) * P, t * P:(t + 1) * P]
        return out

    kcWhTb = stat_b(kcWh)
    kcCTb = stat_b(kcC)
    kcWhT8 = stat_8(kcWh)
    kcCT8 = stat_8(kcC)
    sCb = np.zeros((P, NBF), dtype=NP_BF16)
    for kk in range(NBF):
        sCb[:, kk] = Cs[kk * P:(kk + 1) * P]
    sC8 = np.zeros((P, NPAIR * 32), dtype=NP_FP8)
    for pr in range(NPAIR):
        for i in range(2):
            kk = NBF + 2 * pr + i
            sC8[:, pr * 32 + i * 16] = Cs[kk * P:(kk + 1) * P]

    EmB = np.zeros((P, 2 * 256), dtype=NP_BF16)
    for cc in range(2):
        for d in range(2):
            EmB[:, cc * 256 + d * P:cc * 256 + (d + 1) * P] = \
                E_[cc * P:(cc + 1) * P, d * P:(d + 1) * P]
    rd_w = np.asarray(rd_w, dtype=np.float64)
    rdwB = np.zeros((P, 4 * 256), dtype=NP_BF16)
    for dd in range(4):
        for oo in range(2):
            rdwB[:, dd * 256 + oo * P:dd * 256 + (oo + 1) * P] = \
                rd_w[oo * P:(oo + 1) * P, dd * P:(dd + 1) * P].T

    shared = {"kcWhT8": kcWhT8, "kcCT8": kcCT8, "sC8": sC8,
              "kcWhTb": kcWhTb, "kcCTb": kcCTb, "sCb": sCb,
              "EmB": EmB, "rdwB": rdwB, "scal": scal}
    maps = []
    for cidx in range(NCORES):
        sl = slice(cidx * ROWS, (cidx + 1) * ROWS)
        perm = perms[cidx]
        rowB_c = np.zeros((1, M), dtype=NP_BF16)
        rowB_c[0, :ROWS] = Brow[sl][perm]
        rowB_c[0, ROWS:] = np.float32(SCALE * np.exp(-c))
        exTb_c = np.zeros((P, 2 * M), dtype=NP_BF16)
        exv = ex[sl].astype(f)[perm]                           # [ROWS, 256]
        exTb_c[:, :ROWS] = exv[:, :P].T
        exTb_c[:, M:M + ROWS] = exv[:, P:].T
        # adj: sorted kc cols, sorted-exercise rows, transpose, chunk
        As = np.zeros((M, NKC), dtype=f)
        real = order < adj.shape[1]
        As[:ROWS, real] = np.asarray(adj[sl], dtype=f)[perm][:, order[real]]
        At = As.T.reshape(KCH, P, M)                           # [kk, p, i]
        m_c = {"rowB": rowB_c, "exTb": exTb_c, **shared}
        for g in range(8):
            ag = np.zeros((P, 2 * M), dtype=NP_FP8)
            for o in range(2):
                ag[:, o * M:(o + 1) * M] = At[g * 2 + o]
            m_c[f"adjg{g}"] = ag
        maps.append(m_c)
    return maps, np.asarray(rd_b, dtype=np.float64), tuple(int(t) for t in Ts), perms


def kernel(exercise_h, kc_h, adj, W1, E, a, rd_w, rd_b):
    maps, rdb, Ts, perms = _prep(exercise_h, kc_h, adj, W1, E, a, rd_w, rd_b)
    nc = _get_program(Ts)
    res = run_bass_kernel_spmd(nc, maps, list(range(NCORES))).results
    out = np.empty((N_E, D), dtype=np.float32)
    for cidx in range(NCORES):
        outBv = np.asarray(res[cidx]["outB"]).astype(np.float64)
        s = np.asarray(res[cidx]["srow"]).astype(np.float64)[0, :ROWS]
        A = outBv.reshape(P, M, 2)
        raw = np.concatenate([A[:, :ROWS, 0].T, A[:, :ROWS, 1].T], axis=1)
        o = raw / s[:, None] + rdb[None, :]
        o = np.where(o > 0, o, np.expm1(np.minimum(o, 0)))
        inv = np.empty(ROWS, dtype=np.int64)
        inv[perms[cidx]] = np.arange(ROWS)
        out[cidx * ROWS:(cidx + 1) * ROWS] = o[inv].astype(np.float32)
    return out


# revision 18
# speedup vs baseline: 1.3094x; 1.0322x over previous
"""GAT-style graph encoder on 8 trn2 NeuronCores — v3.

Reference (per exercise i over kc nodes j):
    kc_Wh = kc_h @ W1; ex_Wh = ex_h @ W1
    e[i,j] = leaky_relu(u_i + v_j, 0.2),  u = ex_Wh@a1, v = kc_Wh@a2
    att = softmax(where(adj>0, e, -9e15), axis=1)
    new_kc = att @ kc_Wh; ex_Eh = ex_h @ E
    out = elu(concat([new_kc, new_kc*ex_Eh]) @ rd_w.T + rd_b)

Strategy (row-shard exercises over 8 cores, 1250 rows -> padded 1280):
The pre-activation logit is separable (u_i + v_j), so with the softmax shift
r_i = u_i + c (softmax is invariant to any per-row scale) the masked exp
factors into rank-1 products:
    p[j,i] = adj * max(C'_j, D_j * B'_i),  C' = e^{v-c}, D = e^{0.2 v},
    B' = e^{-0.8 u - c}    (all host-computed rows; exact algebra).
kc nodes are host-sorted by v (descending), exercises are host-sorted by u
(descending, per core).  Then for each kc chunk there is a column prefix
t_kk = #{i : u_i >= -min_j v_j} where the positive branch wins for EVERY
(j,i) pair, i.e. p = adj * C'_j exactly.  For that prefix the aggregation is
a plain matmul with adj itself as the moving tensor and kcWh*C' folded into
the stationary - no elementwise work at all.  Only the column suffix needs
the two elementwise passes (a 4x DVE tensor_scalar for q and a mask multiply
split across DVE/Pool).  The 4 top (high-v) chunks aggregate in bf16; the 12
tail chunks use fp8e4 DoubleRow matmuls (2 k-tiles/instr at 0.5 cyc/row).
Readout runs in bf16.  The per-row softmax division, +rd_b and elu are
applied on the host during unshard (per-row scalar epilogue).
"""

import ml_dtypes
import numpy as np

import concourse.bacc as bacc
import concourse.bass as bass
import concourse.mybir as mybir
from concourse.alu_op_type import AluOpType
from concourse.bass_utils import run_bass_kernel_spmd
from concourse.tile import TileContext

F32 = mybir.dt.float32
BF16 = mybir.dt.bfloat16
FP8 = mybir.dt.float8e4
DR = mybir.MatmulPerfMode.DoubleRow

P = 128
D = 256
NKC = 2048
KCH = 16                    # kc chunks
NBF = 4                     # leading (high-v) chunks aggregated in bf16
NPAIR = (KCH - NBF) // 2    # fp8 DoubleRow chunk pairs
M = 1280                    # padded exercise rows per core
MBS = (512, 512, 256)
MOFF = (0, 512, 1024)
NCORES = 8
ROWS = 1250
N_E = 10000
SCALE = 128.0               # fp8 range scale folded into B'/C' (cancels in n/s)
# tail chunks whose suffix mask multiply runs on DVE (fp8 out, 1x) vs Pool
DVE_MASK = frozenset((5, 7, 9, 11, 13, 15))

NP_BF16 = ml_dtypes.bfloat16
NP_FP8 = ml_dtypes.float8_e4m3


def _build(Ts):
    """Ts: per-chunk column counts (multiple of 64) where p = adj*C' exactly."""
    nc = bacc.Bacc("TRN2", target_bir_lowering=False, debug=False,
                   num_devices=NCORES)
    adjg = [nc.declare_dram_parameter(f"adjg{g}", [P, 2 * M],
                                      FP8, isOutput=False) for g in range(8)]
    exTb = nc.declare_dram_parameter("exTb", [P, 2 * M], BF16, isOutput=False)
    kcWhT8 = nc.declare_dram_parameter("kcWhT8", [P, NPAIR * 512], FP8,
                                       isOutput=False)
    kcWhTb = nc.declare_dram_parameter("kcWhTb", [P, NBF * 256], BF16,
                                       isOutput=False)
    EmB = nc.declare_dram_parameter("EmB", [P, 2 * 256], BF16, isOutput=False)
    rdwB = nc.declare_dram_parameter("rdwB", [P, 4 * 256], BF16,
                                     isOutput=False)
    rowB = nc.declare_dram_parameter("rowB", [1, M], BF16, isOutput=False)
    scal = nc.declare_dram_parameter("scal", [P, 32], F32, isOutput=False)
    outB = nc.declare_dram_parameter("outB", [P, 2 * M], BF16, isOutput=True)
    srow = nc.declare_dram_parameter("srow", [1, M], F32, isOutput=True)

    Tpair = [min(Ts[NBF + 2 * pr], Ts[NBF + 2 * pr + 1])
             for pr in range(NPAIR)]

    with TileContext(nc) as tc:
        with tc.tile_pool(name="const", bufs=1) as cpool, \
             tc.tile_pool(name="acc_ps", bufs=2, space="PSUM") as apool, \
             tc.tile_pool(name="out_ps", bufs=2, space="PSUM") as opool, \
             tc.tile_pool(name="mwork", bufs=4) as mpool, \
             tc.tile_pool(name="post", bufs=2) as qpool:
            # ---- const loads: q-gating rows, then aggregation stationaries
            # and adj groups (unblock PE A-matmuls early), then post-stage data
            scal_sb = cpool.tile([P, 32], F32, tag="scal")
            nc.sync.dma_start(out=scal_sb[:], in_=scal[:, :])
            rowB_sb = cpool.tile([1, M], BF16, tag="rowB")
            nc.sync.dma_start(out=rowB_sb[:], in_=rowB[:, :])
            kcb_sb = cpool.tile([P, NBF * 256], BF16, tag="kcWhTb")
            nc.sync.dma_start(out=kcb_sb[:], in_=kcWhTb[:, :])
            # adj pair-tiles loaded big-mask-suffix first, interleaved with
            # stationaries/post-stage consts; index = pair position (kk//2)
            ADJ_ORDER = (7, 0, 6, 1, 5, 2, 4, 3)
            adj_sb = [None] * 8
            for step, g in enumerate(ADJ_ORDER):
                t = cpool.tile([P, 2, M], FP8, tag=f"adjg{g}", name=f"adjg{g}")
                nc.sync.dma_start(out=t[:], in_=adjg[g][:, :])
                adj_sb[g] = t
                if step == 0:
                    # 4-D: [p, (pair,target), ktile=2, m] for DoubleRow APs
                    kc8_sb = cpool.tile([P, NPAIR * 2, 2, P], FP8, tag="kcWhT8")
                    nc.sync.dma_start(out=kc8_sb[:], in_=kcWhT8[:, :])
                if step == 2:
                    exT_sb = cpool.tile([P, 2 * M], BF16, tag="exTb")
                    nc.sync.dma_start(out=exT_sb[:], in_=exTb[:, :])
                    EmB_sb = cpool.tile([P, 2 * 256], BF16, tag="EmB")
                    nc.sync.dma_start(out=EmB_sb[:], in_=EmB[:, :])
                if step == 4:
                    rdw_sb = cpool.tile([P, 4 * 256], BF16, tag="rdwB")
                    nc.sync.dma_start(out=rdw_sb[:], in_=rdwB[:, :])

            ones1b = cpool.tile([1, P], BF16, tag="ones1b")
            nc.vector.memset(ones1b[:], 1.0)
            onesb = cpool.tile([P, 1], BF16, tag="onesb")
            nc.vector.memset(onesb[:], 1.0)
            ones8 = cpool.tile([P, 2, 16], FP8, tag="ones8")
            nc.vector.memset(ones8[:], 1.0)
            zerob = cpool.tile([P, P], BF16, tag="zerob")
            nc.vector.memset(zerob[:], 0.0)

            Bb = cpool.tile([P, M], BF16, tag="Bb")          # B' broadcast
            exEhT = [cpool.tile([P, M], BF16, tag=f"exEhT{d}", name=f"exEhT{d}")
                     for d in (0, 1)]
            outB_sb = cpool.tile([P, 2 * M], BF16, tag="outB_sb")
            srow_sb = cpool.tile([1, M], F32, tag="srow_sb")

            # ---- setup: B' broadcast + ex_Eh (psum shared with readout pool)
            for b in range(3):
                ms = slice(MOFF[b], MOFF[b] + MBS[b])
                ps = opool.tile([P, MBS[b]], F32, tag="raw", name=f"bb_ps{b}")
                nc.tensor.matmul(ps[:], ones1b[:], rowB_sb[:, ms],
                                 start=True, stop=True)
                nc.scalar.copy(Bb[:, ms], ps[:])
            for d in range(2):
                for b in range(3):
                    ms = slice(MOFF[b], MOFF[b] + MBS[b])
                    ps = opool.tile([P, MBS[b]], F32, tag="raw",
                                    name=f"eh_ps{d}_{b}")
                    for c in range(2):
                        nc.tensor.matmul(
                            ps[:],
                            EmB_sb[:, c * 256 + d * P:c * 256 + (d + 1) * P],
                            exT_sb[:, c * M + MOFF[b]:c * M + MOFF[b] + MBS[b]],
                            start=(c == 0), stop=(c == 1))
                    nc.scalar.copy(exEhT[d][:, ms], ps[:])

            # ---- suffix-only masked-exp (cols >= T of each chunk)
            def adjsl(kk, lo, hi):
                g, o = divmod(kk, 2)
                return adj_sb[g][:, o, lo:hi]

            def q_of(kk, t0):
                q = mpool.tile([P, M], BF16, tag="q", bufs=4,
                               name=f"q{kk}")
                # (B'_i * D_j) max C'_j : whole unmasked exp in one 4x op
                nc.vector.tensor_scalar(
                    q[:, t0:], Bb[:, t0:], scal_sb[:, 16 + kk:17 + kk],
                    scal_sb[:, kk:kk + 1], AluOpType.mult, AluOpType.max)
                return q

            # q rows (full-width, cheap 4x DVE) in mask-priority order;
            # mask multiplies are emitted per m-block (block-major) so early
            # blocks close their accumulations before the last masks finish
            MASK_ORDER = (14, 15, 0, 1, 12, 13, 2, 3, 10, 11, 4, 5, 8, 9, 6, 7)
            qs, ptmb, ptm8 = {}, [None] * NBF, [None] * NPAIR
            for kk in MASK_ORDER:
                t0 = Ts[kk] if kk < NBF else Tpair[(kk - NBF) // 2]
                if t0 >= M:
                    continue
                qs[kk] = q_of(kk, t0)
                if kk < NBF:
                    ptmb[kk] = mpool.tile([P, M], BF16, tag="ptmb", bufs=NBF,
                                          name=f"ptmb{kk}")
                else:
                    pr = (kk - NBF) // 2
                    if ptm8[pr] is None:
                        ptm8[pr] = mpool.tile([P, 2, M], FP8, tag="ptm8",
                                              bufs=NPAIR, name=f"ptm8_{pr}")
            for b in range(3):
                for kk in MASK_ORDER:
                    t0 = Ts[kk] if kk < NBF else Tpair[(kk - NBF) // 2]
                    lo = max(t0, MOFF[b])
                    hi = MOFF[b] + MBS[b]
                    if lo >= hi:
                        continue
                    if kk < NBF:
                        nc.vector.tensor_mul(ptmb[kk][:, lo:hi],
                                             qs[kk][:, lo:hi],
                                             adjsl(kk, lo, hi))
                    else:
                        pr, h = divmod(kk - NBF, 2)
                        eng = nc.vector if kk in DVE_MASK else nc.gpsimd
                        eng.tensor_mul(ptm8[pr][:, h, lo:hi],
                                       qs[kk][:, lo:hi], adjsl(kk, lo, hi))

            # ---- per m-block aggregation + readout
            for b in range(3):
                mb = MBS[b]
                ms = slice(MOFF[b], MOFF[b] + mb)
                n0 = apool.tile([P, mb], F32, tag="n0")
                n1 = apool.tile([P, mb], F32, tag="n1")
                sS = apool.tile([1, mb], F32, tag="sS")

                # start=True zeroes the whole 2KB psum bank, so the group
                # opener must be a full-width matmul emitted first: prefer a
                # chunk whose exact-C prefix covers the block (adj-only dep,
                # lets PE start without waiting for masks), else open with a
                # zero-stationary matmul on an always-ready moving tile.
                starter = None
                for kk in range(NBF):
                    if Ts[kk] - MOFF[b] >= mb:
                        starter = kk
                        break
                if starter is None:
                    nc.tensor.matmul(n0[:], zerob[:], Bb[:, ms],
                                     start=True, stop=False,
                                     skip_group_check=True)
                    nc.tensor.matmul(n1[:], zerob[:], Bb[:, ms],
                                     start=True, stop=False,
                                     skip_group_check=True)
                    nc.tensor.matmul(sS[:], zerob[:, 0:1], Bb[:, ms],
                                     start=True, stop=False,
                                     skip_group_check=True)
                order = ([starter] + [k for k in range(NBF) if k != starter]
                         if starter is not None else list(range(NBF)))
                for kk in order:
                    aw = min(max(Ts[kk] - MOFF[b], 0), mb)
                    st = (starter is not None and kk == starter)
                    if aw > 0:
                        asl = adjsl(kk, MOFF[b], MOFF[b] + aw)
                        nc.tensor.matmul(
                            n0[:, 0:aw], kcb_sb[:, kk * 256:kk * 256 + P],
                            asl, start=st, stop=False, skip_group_check=True)
                        nc.tensor.matmul(
                            n1[:, 0:aw], kcb_sb[:, kk * 256 + P:(kk + 1) * 256],
                            asl, start=st, stop=False, skip_group_check=True)
                        nc.tensor.matmul(
                            sS[:, 0:aw], onesb[:],
                            asl, start=st, stop=False, skip_group_check=True)
                    if aw < mb:
                        pm = ptmb[kk][:, MOFF[b] + aw:MOFF[b] + mb]
                        nc.tensor.matmul(
                            n0[:, aw:mb], kcb_sb[:, kk * 256:kk * 256 + P],
                            pm, start=st, stop=False, skip_group_check=True)
                        nc.tensor.matmul(
                            n1[:, aw:mb], kcb_sb[:, kk * 256 + P:(kk + 1) * 256],
                            pm, start=st, stop=False, skip_group_check=True)
                        nc.tensor.matmul(
                            sS[:, aw:mb], onesb[:],
                            pm, start=st, stop=False, skip_group_check=True)
                # fp8 DoubleRow pairs
                for pr in range(NPAIR):
                    aw = min(max(Tpair[pr] - MOFF[b], 0), mb)
                    sp = (pr == NPAIR - 1)
                    if aw > 0:
                        adjpair = adj_sb[(NBF + 2 * pr) // 2][:, :, MOFF[b]:MOFF[b] + aw]
                        nc.tensor.matmul(
                            n0[:, 0:aw], kc8_sb[:, 2 * pr, :, :], adjpair,
                            start=False, stop=sp and aw >= mb,
                            perf_mode=DR, skip_group_check=True)
                        nc.tensor.matmul(
                            n1[:, 0:aw], kc8_sb[:, 2 * pr + 1, :, :], adjpair,
                            start=False, stop=sp and aw >= mb,
                            perf_mode=DR, skip_group_check=True)
                        nc.tensor.matmul(
                            sS[:, 0:aw], ones8[:, :, 0:1], adjpair,
                            start=False, stop=sp and aw >= mb,
                            perf_mode=DR, skip_group_check=True)
                    if aw < mb:
                        pm = ptm8[pr][:, :, MOFF[b] + aw:MOFF[b] + mb]
                        nc.tensor.matmul(
                            n0[:, aw:mb], kc8_sb[:, 2 * pr, :, :], pm,
                            start=False, stop=sp, perf_mode=DR,
                            skip_group_check=True)
                        nc.tensor.matmul(
                            n1[:, aw:mb], kc8_sb[:, 2 * pr + 1, :, :], pm,
                            start=False, stop=sp, perf_mode=DR,
                            skip_group_check=True)
                        nc.tensor.matmul(
                            sS[:, aw:mb], ones8[:, :, 0:1], pm,
                            start=False, stop=sp, perf_mode=DR,
                            skip_group_check=True)

                # ---- post: features, readout, stage out
                nc.vector.tensor_copy(srow_sb[:, ms], sS[:])
                ncf = []
                for t in range(2):
                    nt = qpool.tile([P, mb], BF16, tag=f"nc{t}", name=f"nc{t}")
                    nc.scalar.copy(nt[:], (n0 if t == 0 else n1)[:])
                    ncf.append(nt)
                tf = []
                for t in range(2):
                    tt = qpool.tile([P, mb], BF16, tag=f"t{t}", name=f"tt{t}")
                    nc.vector.tensor_mul(tt[:], ncf[t][:], exEhT[t][:, ms])
                    tf.append(tt)
                feats = [ncf[0], ncf[1], tf[0], tf[1]]
                for oo in range(2):
                    raw = opool.tile([P, mb], F32, tag="raw")
                    for dd in range(4):
                        nc.tensor.matmul(
                            raw[:], rdw_sb[:, dd * 256 + oo * P:dd * 256 + (oo + 1) * P],
                            feats[dd][:], start=(dd == 0), stop=(dd == 3))
                    # stage to outB interleaved (col 2i+oo) for one DMA/block
                    nc.scalar.copy(
                        outB_sb[:, 2 * MOFF[b] + oo:2 * (MOFF[b] + mb):2],
                        raw[:])
                nc.sync.dma_start(
                    out=outB[:, 2 * MOFF[b]:2 * (MOFF[b] + mb)],
                    in_=outB_sb[:, 2 * MOFF[b]:2 * (MOFF[b] + mb)])
            nc.sync.dma_start(out=srow[:, :], in_=srow_sb[:])
    nc.finalize()
    return nc


_PROGRAMS = {}


def _get_program(Ts):
    key = tuple(Ts)
    if key not in _PROGRAMS:
        _PROGRAMS[key] = _build(key)
    return _PROGRAMS[key]


def _prep(exercise_h, kc_h, adj, W1, E, a, rd_w, rd_b):
    f = np.float32
    ex = np.asarray(exercise_h, dtype=np.float64)
    kc = np.asarray(kc_h, dtype=np.float64)
    W1 = np.asarray(W1, dtype=np.float64)
    E_ = np.asarray(E, dtype=np.float64)
    a = np.asarray(a, dtype=np.float64)
    a1, a2 = a[:D, 0], a[D:, 0]

    u = ex @ (W1 @ a1)                        # [N_E]
    vp = np.full(NKC, -60.0)
    vp[:kc.shape[0]] = kc @ (W1 @ a2)
    order = np.argsort(-vp, kind="stable")
    vs = vp[order]
    vmax = vs[0]
    c = float((np.maximum(u + vmax, 0.2 * (u + vmax)) - u).max())

    Brow = (SCALE * np.exp(-0.8 * u - c)).astype(f)            # [N_E]
    Cs = (SCALE * np.exp(vs - c)).astype(f)                    # [NKC]
    Ds = np.exp(0.2 * vs).astype(f)                            # [NKC]
    scal = np.zeros((P, 32), dtype=f)
    scal[:, :16] = Cs.reshape(KCH, P).T
    scal[:, 16:] = Ds.reshape(KCH, P).T

    # per-core exercise sort by u (descending) + per-chunk exact-C prefix
    perms = []
    Ts = np.full(KCH, M, dtype=np.int64)
    vlo = vs.reshape(KCH, P).min(axis=1)                       # chunk min v
    for cidx in range(NCORES):
        uc = u[cidx * ROWS:(cidx + 1) * ROWS]
        perm = np.argsort(-uc, kind="stable")
        perms.append(perm)
        us = uc[perm]
        for kk in range(KCH):
            cnt = int((us >= -vlo[kk]).sum())                  # prefix length
            Ts[kk] = min(Ts[kk], cnt)
    Ts = (Ts // 64) * 64                                       # align, pads are B-cols
    Ts = np.minimum(Ts, ROWS // 64 * 64)

    kcp = np.zeros((NKC, D))
    kcp[:kc.shape[0]] = kc
    kcWh = (kcp[order] @ W1).astype(f)                         # [NKC, D]

    def stat_b(src):
        out = np.zeros((P, NBF * 256), dtype=NP_BF16)
        for kk in range(NBF):
            for t in range(2):
                out[:, kk * 256 + t * P:kk * 256 + (t + 1) * P] = \
                    src[kk * P:(kk + 1) * P, t * P:(t + 1) * P]
        return out

    def stat_8(src):
        s8 = src.astype(NP_FP8)
        out = np.zeros((P, NPAIR * 512), dtype=NP_FP8)
        for pr in range(NPAIR):
            for t in range(2):
                for i in range(2):
                    kk = NBF + 2 * pr + i
                    out[:, pr * 512 + t * 256 + i * P:pr * 512 + t * 256 + (i + 1) * P] = \
                        s8[kk * P:(kk + 1) * P, t * P:(t + 1) * P]
        return out

    kcWhTb = stat_b(kcWh)
    kcWhT8 = stat_8(kcWh)

    EmB = np.zeros((P, 2 * 256), dtype=NP_BF16)
    for cc in range(2):
        for d in range(2):
            EmB[:, cc * 256 + d * P:cc * 256 + (d + 1) * P] = \
                E_[cc * P:(cc + 1) * P, d * P:(d + 1) * P]
    rd_w = np.asarray(rd_w, dtype=np.float64)
    rdwB = np.zeros((P, 4 * 256), dtype=NP_BF16)
    for dd in range(4):
        for oo in range(2):
            rdwB[:, dd * 256 + oo * P:dd * 256 + (oo + 1) * P] = \
                rd_w[oo * P:(oo + 1) * P, dd * P:(dd + 1) * P].T

    shared = {"kcWhT8": kcWhT8, "kcWhTb": kcWhTb,
              "EmB": EmB, "rdwB": rdwB, "scal": scal}
    maps = []
    for cidx in range(NCORES):
        sl = slice(cidx * ROWS, (cidx + 1) * ROWS)
        perm = perms[cidx]
        rowB_c = np.zeros((1, M), dtype=NP_BF16)
        rowB_c[0, :ROWS] = Brow[sl][perm]
        rowB_c[0, ROWS:] = np.float32(SCALE * np.exp(-c))
        exTb_c = np.zeros((P, 2 * M), dtype=NP_BF16)
        exv = ex[sl].astype(f)[perm]                           # [ROWS, 256]
        exTb_c[:, :ROWS] = exv[:, :P].T
        exTb_c[:, M:M + ROWS] = exv[:, P:].T
        # adj: sorted kc cols, sorted-exercise rows, transpose, chunk
        As = np.zeros((M, NKC), dtype=f)
        real = order < adj.shape[1]
        As[:ROWS, real] = np.asarray(adj[sl], dtype=f)[perm][:, order[real]]
        At = As.T.reshape(KCH, P, M)                           # [kk, p, i]
        # adj: prefix cols (exact-C region) carry adj*C'_j, suffix cols
        # (mask region) carry plain {0,1}
        Tpair = [min(Ts[NBF + 2 * pr], Ts[NBF + 2 * pr + 1])
                 for pr in range(NPAIR)]
        m_c = {"rowB": rowB_c, "exTb": exTb_c, **shared}
        for g in range(8):
            ag = np.zeros((P, 2 * M), dtype=NP_FP8)
            for o in range(2):
                kk = g * 2 + o
                tk = Ts[kk] if kk < NBF else Tpair[(kk - NBF) // 2]
                blk = At[kk].copy()
                blk[:, :tk] *= Cs[kk * P:(kk + 1) * P, None]
                ag[:, o * M:(o + 1) * M] = blk
            m_c[f"adjg{g}"] = ag
        maps.append(m_c)
    return maps, np.asarray(rd_b, dtype=np.float64), tuple(int(t) for t in Ts), perms


def kernel(exercise_h, kc_h, adj, W1, E, a, rd_w, rd_b):
    maps, rdb, Ts, perms = _prep(exercise_h, kc_h, adj, W1, E, a, rd_w, rd_b)
    nc = _get_program(Ts)
    res = run_bass_kernel_spmd(nc, maps, list(range(NCORES))).results
    out = np.empty((N_E, D), dtype=np.float32)
    for cidx in range(NCORES):
        outBv = np.asarray(res[cidx]["outB"]).astype(np.float64)
        s = np.asarray(res[cidx]["srow"]).astype(np.float64)[0, :ROWS]
        A = outBv.reshape(P, M, 2)
        raw = np.concatenate([A[:, :ROWS, 0].T, A[:, :ROWS, 1].T], axis=1)
        o = raw / s[:, None] + rdb[None, :]
        o = np.where(o > 0, o, np.expm1(np.minimum(o, 0)))
        inv = np.empty(ROWS, dtype=np.int64)
        inv[perms[cidx]] = np.arange(ROWS)
        out[cidx * ROWS:(cidx + 1) * ROWS] = o[inv].astype(np.float32)
    return out
